# revision 15
# baseline (speedup 1.0000x reference)
"""Bass/Trainium2 kernel for nn_BiChannelAttention (single-query local-window attention).

Math (per batch b, head h, S=2049, window W=256, cutoff=S-W=1793):
  Every in-window position carries the same -1e6 time-mask shift (the reference's
  masked_fill collapses to a uniform constant), which softmax cancels; positions
  before the cutoff are -1e6 relative => weight exactly 0 in fp32. Only the last
  W positions (255 cache rows + the content row) matter.

  Per (b,h) with window rows X [W,128]:
    kq  = 64/sqrt(128) * (Wk Wq^T cnt + Wk bq)   (Wq,Wk host-folded; x64 keeps
          fp8 kq out of subnormals; q.bk is an s-constant, cancels in softmax)
    sc  = X kq                                    (PE matvec, s on partitions)
    a   = exp(sc/64 - pos_param*bucket(s))        (ACT: scale+bias fused)
    av  = X^T a ; z^T = a^T ones                  (PE; z lands b-on-partitions)
    outT = (av^T Wv) * (1/z)[b] + (bv + cnt)^T    (transposed output: per-b 1/z
          becomes a per-partition tensor_scalar -- no broadcast matmul needed)

Precision: window data X ships as fp8(e4m3) in BOTH layouts (natural [s,d] for
the AV stationary, transposed [d,s] for the scores stationary); weights/cnt
bf16, accumulation fp32 in PSUM. Host-simulated rel err vs the fp32 reference:
~9e-4 (gate 2e-2) -- the residual add dilutes attention-path error ~20x.

Perf structure (from NTFF traces): each dma_start costs ~650ns issue plus
serialized per-descriptor dispatch on its HWDGE queue, so inputs ship as ONE
merged byte-buffer DMA per queue (sync + scalar), bitcast into typed regions
on SBUF; the output is built transposed [16,128] so the store is 16
descriptors instead of 128. Stationaries are per-(pair,half) 128-col fp8
tiles (FWL); LDW+MM pairs pipeline at ~26ns. Dummy-matmul warmup keeps the
PE HAM clock warm through the DMA window. DVE ops are ordered so the strict
FIFO never blocks a later head's prerequisites behind an earlier head's tail.

Sharding: tensor-parallel over heads, 2 heads per core x 8 cores.
"""

import sys
import numpy as np

for _p in ("/opt/trn_rl_repo", "/root/.axon_site/_ro/trn_rl_repo"):
    if _p not in sys.path:
        sys.path.insert(0, _p)

import ml_dtypes
import concourse.bass as bass
import concourse.bacc as bacc
import concourse.mybir as mybir
from concourse.tile import TileContext
from concourse.bass_utils import run_bass_kernel_spmd

F32 = mybir.dt.float32
BF16 = mybir.dt.bfloat16
FP8 = mybir.dt.float8e4
U8 = mybir.dt.uint8
NPBF = ml_dtypes.bfloat16
NPF8 = ml_dtypes.float8_e4m3fn
P = 128          # partitions / head_dim
B = 8            # batch
H = 16           # heads total
HPC = 2          # heads per core
NCORES = 8
T = 2048
S = T + 1
W = 256          # local attention window
CUTOFF = S - W   # 1793
NWARM = 30       # PE warmup matmuls (HAM ramp ~3.4us, covers DMA window)
KQS = 512.0      # kq prescale (undone by exp's scale=1/KQS)
WVS = 64.0       # Wv prescale (cancelled exactly by ones8=WVS in the z matmul)

_NC_CACHE = {}

# Buffer A (sync queue), bytes per partition row:
#   [cnt bf16 16 cols = 32 B][per head: M fp8 128 B | wv fp8 128 B]
#   [cf f32 16 B][xt0 fp8 2048 B][xt1 fp8 2048 B]
A_W0 = 2 * HPC * B                             # byte offsets
A_CF = A_W0 + HPC * 2 * P
A_XT0 = A_CF + 16
A_XT1 = A_XT0 + B * W
A_BYTES = A_XT1 + B * W
# Buffer B (scalar queue):
#   [cf f32 4 cols = 16 B: bias0|bias1|vb0|vb1][cntT f32 2x128 cols = 1024 B
#    (per head, partitions 0-7 hold (cnt+bv)^T, rest zero)][xn0 2048][xn1 2048]
CFW = 4
B_CNTT = 0
B_XN0 = B_CNTT + HPC * 4 * P
B_XN1 = B_XN0 + B * W
B_BYTES = B_XN1 + B * W


def _build_nc():
    nc = bacc.Bacc(None, target_bir_lowering=False, debug=False)
    ina_in = nc.declare_dram_parameter("ina", [P, A_BYTES], U8, isOutput=False)
    inb_in = nc.declare_dram_parameter("inb", [P, B_BYTES], U8, isOutput=False)
    out_t = nc.declare_dram_parameter("out", [HPC, B, P], F32, isOutput=True)

    with TileContext(nc) as tc:
        with (
            nc.allow_low_precision(reason="fp8/bf16 pipeline validated vs reference"),
            tc.tile_pool(name="big", bufs=1) as bigp,
            tc.tile_pool(name="small", bufs=8) as spool,
            tc.tile_pool(name="ps_qk", bufs=2, space="PSUM") as psQK,
            tc.tile_pool(name="ps_s", bufs=2, space="PSUM") as psS,
            tc.tile_pool(name="ps_zo", bufs=2, space="PSUM") as psZO,
        ):
            # ---- input DMAs ordered by consumption time: smalls (kq chain)
            # first, then per-head window chunks, cntT (tail-only) last.
            ina = bigp.tile([P, A_BYTES], U8, tag="ina")
            inb = bigp.tile([P, B_BYTES], U8, tag="inb")
            nc.sync.dma_start(out=ina[:, 0:A_XT0], in_=ina_in[:, 0:A_XT0])
            nc.sync.dma_start(out=ina[:, A_XT0:A_XT1],
                              in_=ina_in[:, A_XT0:A_XT1])
            nc.sync.dma_start(out=ina[:, A_XT1:A_BYTES],
                              in_=ina_in[:, A_XT1:A_BYTES])
            nc.scalar.dma_start(out=inb[:, B_XN0:B_XN1],
                                in_=inb_in[:, B_XN0:B_XN1])
            nc.scalar.dma_start(out=inb[:, B_XN1:B_BYTES],
                                in_=inb_in[:, B_XN1:B_BYTES])
            nc.scalar.dma_start(out=inb[:, 0:B_XN0], in_=inb_in[:, 0:B_XN0])

            cnt_bf = ina[:, 0:HPC * B].bitcast(FP8)  # fp8 to match M
            Mw = [ina[:, A_W0 + j * 2 * P:A_W0 + j * 2 * P + P].bitcast(FP8)
                  for j in range(HPC)]
            wv = [ina[:, A_W0 + j * 2 * P + P:A_W0 + j * 2 * P + 2 * P]
                  .bitcast(FP8) for j in range(HPC)]
            xt = [ina[:, A_XT0:A_XT0 + B * W].bitcast(FP8),
                  ina[:, A_XT1:A_XT1 + B * W].bitcast(FP8)]
            cf = ina[:, A_CF:A_CF + 16].bitcast(F32)
            cntT = [inb[:, B_CNTT + j * 4 * P:B_CNTT + (j + 1) * 4 * P]
                    .bitcast(F32) for j in range(HPC)]  # rows 0-7 used
            xn = [inb[:, B_XN0:B_XN0 + B * W].bitcast(FP8),
                  inb[:, B_XN1:B_XN1 + B * W].bitcast(FP8)]

            ones8 = spool.tile([P, 1], FP8, tag="ones8")
            nc.vector.memset(ones8[:, :], WVS)

            # ---- PE warmup while the DMAs land
            wdat = spool.tile([P, P], BF16, tag="warm")
            nc.vector.memset(wdat[:, :], 0.0)
            wps = psZO.tile([P, P], F32, tag="zo")
            for i in range(NWARM):
                nc.tensor.matmul(wps[:, :], wdat[:, :], wdat[:, :],
                                 start=True, stop=True)

            # ---- phase 1: kq for both heads (keeps DVE FIFO unblocked)
            kq_sb = []
            for j in range(HPC):
                kq_ps = psQK.tile([P, B], F32, tag="qk")
                nc.tensor.matmul(kq_ps[:, :], Mw[j], cnt_bf[:, j * B:(j + 1) * B],
                                 start=True, stop=True)
                k_sb = spool.tile([P, B], FP8, tag=f"kqsb{j}")
                nc.vector.tensor_scalar_add(k_sb[:, :], kq_ps[:, :],
                                            cf[:, 2 + j:3 + j])
                kq_sb.append(k_sb)

            # ---- phase 2: scores + exp, batched across heads per s-half
            # (both heads share the same per-partition bias for a given half,
            # so each half needs only ONE activation over [128,16])
            att = []
            for h in range(2):
                sch = psS.tile([P, HPC * B], F32, tag="sc")
                for j in range(HPC):
                    for b in range(B):
                        nc.tensor.matmul(sch[:, j * B + b:j * B + b + 1],
                                         xt[j][:, b * W + h * P:b * W + (h + 1) * P],
                                         kq_sb[j][:, b:b + 1],
                                         start=True, stop=True)
                a_h = spool.tile([P, HPC * B], FP8, tag=f"att{h}")
                nc.scalar.activation(a_h[:, :], sch[:, :],
                                     mybir.ActivationFunctionType.Exp,
                                     bias=cf[:, h:h + 1], scale=1.0 / KQS)
                att.append(a_h)

            # ---- phase 3: av + z^T; a_sb cols for head j live at j*B+b in
            # each half tile. Tails split across engines: h0 via GPSIMD, h1
            # via DVE, so the two normalize/residual chains run in parallel.
            av_sb, rec_t = [], []
            for j in range(HPC):
                pool = psQK if j == 0 else psS
                av = pool.tile([P, B], F32, tag="qk" if j == 0 else "sc")
                for b in range(B):
                    nc.tensor.matmul(av[:, b:b + 1],
                                     xn[j][:, b * W:b * W + P],
                                     att[0][:, j * B + b:j * B + b + 1],
                                     start=True, stop=False)
                    nc.tensor.matmul(av[:, b:b + 1],
                                     xn[j][:, b * W + P:b * W + 2 * P],
                                     att[1][:, j * B + b:j * B + b + 1],
                                     start=False, stop=True)
                zt_ps = pool.tile([B, 1], F32, tag="qk" if j == 0 else "sc")
                nc.tensor.matmul(zt_ps[:, :], att[0][:, j * B:(j + 1) * B],
                                 ones8, start=True, stop=False)
                nc.tensor.matmul(zt_ps[:, :], att[1][:, j * B:(j + 1) * B],
                                 ones8, start=False, stop=True)
                a_v = spool.tile([P, B], FP8, tag=f"avsb{j}")
                nc.vector.tensor_copy(a_v[:, :], av[:, :])
                r_t = spool.tile([B, 1], F32, tag=f"rec{j}")
                nc.vector.reciprocal(r_t[:, :], zt_ps[:, :])
                av_sb.append(a_v)
                rec_t.append(r_t)

            # ---- phase 4: o^T = (WVS*Wv)^T av, normalize by 1/(WVS*z), +cnt
            ot_ps = []
            for j in range(HPC):
                o_p = psZO.tile([B, P], F32, tag="zo")
                nc.tensor.matmul(o_p[:, :], av_sb[j][:, :], wv[j],
                                 start=True, stop=True)
                ot_ps.append(o_p)
            for j in range(HPC):
                tmp_t = spool.tile([B, P], F32, tag=f"tmp{j}")
                fin_t = spool.tile([B, P], F32, tag=f"fin{j}")
                if j == 0:
                    # h0 tail on ACT (PSUM-capable) + GPSIMD; h1 on DVE --
                    # the two chains run on disjoint engines in parallel
                    nc.scalar.activation(tmp_t[:, :], ot_ps[j][:, :],
                                         mybir.ActivationFunctionType.Copy,
                                         bias=0.0, scale=rec_t[j][:, :])
                    nc.gpsimd.tensor_add(fin_t[:, :], tmp_t[:, :],
                                         cntT[j][0:B, :])
                else:
                    nc.vector.tensor_scalar(tmp_t[:, :], ot_ps[j][:, :],
                                            rec_t[j][:, :], None,
                                            mybir.AluOpType.mult)
                    nc.vector.tensor_add(fin_t[:, :], tmp_t[:, :],
                                         cntT[j][0:B, :])
                if j == 0:
                    nc.scalar.dma_start(out=out_t[0], in_=fin_t[:, :])
                else:
                    nc.sync.dma_start(out=out_t[1], in_=fin_t[:, :])
    nc.finalize()
    return nc


def _get_nc():
    if "nc" not in _NC_CACHE:
        _NC_CACHE["nc"] = _build_nc()
    return _NC_CACHE["nc"]


def _pos_bias_f32():
    """t5_position_bucket exactly as the reference computes it, sliced to the
    window."""
    if "pos" not in _NC_CACHE:
        import jax.numpy as jnp
        NUM_BUCKETS, MAX_DISTANCE = 32, 128
        n = (S - 1) - jnp.arange(S)
        max_exact = NUM_BUCKETS // 2
        is_small = n < max_exact
        large = max_exact + (
            jnp.log(jnp.maximum(n, 1).astype(jnp.float32) / max_exact)
            / np.log(MAX_DISTANCE / max_exact)
            * (NUM_BUCKETS - max_exact)
        ).astype(jnp.int32)
        large = jnp.minimum(large, NUM_BUCKETS - 1)
        pos = jnp.where(is_small, n, large).astype(jnp.float32)
        _NC_CACHE["pos"] = np.asarray(pos)[CUTOFF:]  # [W]
    return _NC_CACHE["pos"]


def kernel(**inputs) -> np.ndarray:
    t = int(np.asarray(inputs["t"]))
    assert t == T, f"kernel hardcoded for t={T}, got {t}"
    content_t = np.asarray(inputs["content_t"], dtype=np.float32)
    cache = np.asarray(inputs["cache"], dtype=np.float32)
    Wq = np.asarray(inputs["Wq"], dtype=np.float32)
    bq = np.asarray(inputs["bq"], dtype=np.float32)
    Wk = np.asarray(inputs["Wk"], dtype=np.float32)
    Wv = np.asarray(inputs["Wv"], dtype=np.float32)
    bv = np.asarray(inputs["bv"], dtype=np.float32)
    pos_param = np.float32(np.asarray(inputs["pos_param"]))
    # time_mask: uniform -1e6 shift in-window (softmax-invariant); bk: adds an
    # s-constant q.bk to every in-window score (softmax-invariant). Both dropped.

    pos = _pos_bias_f32()                                   # [W]
    posb = (-pos_param * pos).astype(np.float32)            # [W]
    c = np.float32(KQS / np.sqrt(128.0))
    WVSf = np.float32(WVS)

    win = cache[:, CUTOFF:T, :].reshape(B, W - 1, H, P)     # [B, 255, H, 128]
    cnt_h = content_t.reshape(B, H, P)                      # [B, H, 128]

    in_maps = []
    for co in range(NCORES):
        h0 = HPC * co
        ina = np.zeros((P, A_BYTES), np.uint8)
        inb = np.zeros((P, B_BYTES), np.uint8)
        ina[:, 0:HPC * B] = (
            cnt_h[:, h0:h0 + HPC, :].transpose(2, 1, 0).reshape(P, HPC * B)
            .astype(NPF8).view(np.uint8)
        )
        for j in range(HPC):
            base = A_W0 + j * 2 * P
            h = h0 + j
            ina[:, base:base + P] = (
                (c * (Wq[h] @ Wk[h].T)).astype(NPF8).view(np.uint8)
            )
            ina[:, base + P:base + 2 * P] = (
                (np.float32(WVS) * Wv[h]).astype(NPF8).view(np.uint8)
            )
        for j in range(HPC):
            xwin = np.concatenate(
                [win[:, :, h0 + j, :], cnt_h[:, None, h0 + j, :]], axis=1
            )                                               # [B, 256, 128] f32
            xwb = xwin.astype(NPF8)
            off = A_XT0 if j == 0 else A_XT1
            ina[:, off:off + B * W] = (
                xwb.transpose(2, 0, 1).reshape(P, B * W).view(np.uint8)
            )
            offn = B_XN0 if j == 0 else B_XN1
            inb[:, offn:offn + B * W] = (
                xwb.reshape(B, 2, P, P).transpose(2, 0, 1, 3)
                .reshape(P, B * W).view(np.uint8)
            )
        cfh = np.zeros((P, CFW), np.float32)
        cfh[:, 0] = posb[0:P]
        cfh[:, 1] = posb[P:2 * P]
        for j in range(HPC):
            cfh[:, 2 + j] = c * (Wk[h0 + j] @ bq[h0 + j])
        ina[:, A_CF:A_CF + 16] = cfh.view(np.uint8)
        for j in range(HPC):
            cntT = np.zeros((P, P), np.float32)
            cntT[0:B, :] = cnt_h[:, h0 + j, :] + bv[None, h0 + j, :]
            inb[:, B_CNTT + j * 4 * P:B_CNTT + (j + 1) * 4 * P] = (
                cntT.view(np.uint8)
            )
        in_maps.append({"ina": ina, "inb": inb})

    nc = _get_nc()
    res = run_bass_kernel_spmd(nc, in_maps, list(range(NCORES)), **_RUN_KWARGS)
    _NC_CACHE["last_results"] = res
    outs = np.stack([np.asarray(res.results[co]["out"]) for co in range(NCORES)])
    # outs: [core, j, b, d] -> out_full[b, (2c+j)*128 + d]
    out_full = outs.transpose(2, 0, 1, 3).reshape(B, H * P)
    return np.ascontiguousarray(out_full, dtype=np.float32)


_RUN_KWARGS = {}  # test harness may set {"trace": True, "tmpdir": ...}


# revision 16
# speedup vs baseline: 1.1560x; 1.1560x over previous
"""Bass/Trainium2 kernel for nn_BiChannelAttention (single-query local-window attention).

Math (per batch b, head h, S=2049, window W=256, cutoff=S-W=1793):
  Every in-window position carries the same -1e6 time-mask shift (the reference's
  masked_fill collapses to a uniform constant), which softmax cancels; positions
  before the cutoff are -1e6 relative => weight exactly 0 in fp32. Only the last
  W positions (255 cache rows + the content row) matter.

  Per (b,h) with window rows X [W,128]:
    kq  = 64/sqrt(128) * (Wk Wq^T cnt + Wk bq)   (Wq,Wk host-folded; x64 keeps
          fp8 kq out of subnormals; q.bk is an s-constant, cancels in softmax)
    sc  = X kq                                    (PE matvec, s on partitions)
    a   = exp(sc/64 - pos_param*bucket(s))        (ACT: scale+bias fused)
    av  = X^T a ; z^T = a^T ones                  (PE; z lands b-on-partitions)
    outT = (av^T Wv) * (1/z)[b] + (bv + cnt)^T    (transposed output: per-b 1/z
          becomes a per-partition tensor_scalar -- no broadcast matmul needed)

Precision: window data X ships as fp8(e4m3) in BOTH layouts (natural [s,d] for
the AV stationary, transposed [d,s] for the scores stationary); weights/cnt
bf16, accumulation fp32 in PSUM. Host-simulated rel err vs the fp32 reference:
~9e-4 (gate 2e-2) -- the residual add dilutes attention-path error ~20x.

Perf structure (from NTFF traces): each dma_start costs ~650ns issue plus
serialized per-descriptor dispatch on its HWDGE queue, so inputs ship as ONE
merged byte-buffer DMA per queue (sync + scalar), bitcast into typed regions
on SBUF; the output is built transposed [16,128] so the store is 16
descriptors instead of 128. Stationaries are per-(pair,half) 128-col fp8
tiles (FWL); LDW+MM pairs pipeline at ~26ns. Dummy-matmul warmup keeps the
PE HAM clock warm through the DMA window. DVE ops are ordered so the strict
FIFO never blocks a later head's prerequisites behind an earlier head's tail.

Sharding: tensor-parallel over heads, 2 heads per core x 8 cores.
"""

import sys
import numpy as np

for _p in ("/opt/trn_rl_repo", "/root/.axon_site/_ro/trn_rl_repo"):
    if _p not in sys.path:
        sys.path.insert(0, _p)

import ml_dtypes
import concourse.bass as bass
import concourse.bacc as bacc
import concourse.mybir as mybir
from concourse.tile import TileContext
from concourse.bass_utils import run_bass_kernel_spmd

F32 = mybir.dt.float32
BF16 = mybir.dt.bfloat16
FP8 = mybir.dt.float8e4
U8 = mybir.dt.uint8
NPBF = ml_dtypes.bfloat16
NPF8 = ml_dtypes.float8_e4m3fn
P = 128          # partitions / head_dim
B = 8            # batch
H = 16           # heads total
HPC = 2          # heads per core
NCORES = 8
T = 2048
S = T + 1
W = 256          # local attention window
CUTOFF = S - W   # 1793
NWARM = 30       # PE warmup matmuls (HAM ramp ~3.4us, covers DMA window)
KQS = 512.0      # kq prescale (undone by exp's scale=1/KQS)
WVS = 64.0       # Wv prescale (cancelled exactly by ones8=WVS in the z matmul)

_NC_CACHE = {}

# Buffer A (sync queue), bytes per partition row:
#   [cnt bf16 16 cols = 32 B][per head: M fp8 128 B | wv fp8 128 B]
#   [cf f32 16 B][xt0 fp8 2048 B][xt1 fp8 2048 B]
A_W0 = 2 * HPC * B                             # byte offsets
A_CF = A_W0 + HPC * 2 * P
A_XT0 = A_CF + 16
A_XT1 = A_XT0 + B * W
A_BYTES = A_XT1 + B * W
# Buffer B (scalar queue):
#   [cf f32 4 cols = 16 B: bias0|bias1|vb0|vb1][cntT f32 2x128 cols = 1024 B
#    (per head, partitions 0-7 hold (cnt+bv)^T, rest zero)][xn0 2048][xn1 2048]
CFW = 4
B_CNTT = 0
B_XN0 = B_CNTT + HPC * 4 * P
B_XN1 = B_XN0 + B * W
B_BYTES = B_XN1 + B * W


def _build_nc():
    nc = bacc.Bacc(None, target_bir_lowering=False, debug=False)
    ina_in = nc.declare_dram_parameter("ina", [P, A_BYTES], U8, isOutput=False)
    inb_in = nc.declare_dram_parameter("inb", [P, B_BYTES], U8, isOutput=False)
    out_t = nc.declare_dram_parameter("out", [HPC, B, P], F32, isOutput=True)

    with TileContext(nc) as tc:
        with (
            nc.allow_low_precision(reason="fp8/bf16 pipeline validated vs reference"),
            tc.tile_pool(name="big", bufs=1) as bigp,
            tc.tile_pool(name="small", bufs=8) as spool,
            tc.tile_pool(name="ps_qk", bufs=2, space="PSUM") as psQK,
            tc.tile_pool(name="ps_s", bufs=2, space="PSUM") as psS,
            tc.tile_pool(name="ps_zo", bufs=2, space="PSUM") as psZO,
        ):
            # ---- one merged DMA per HWDGE queue (per-DMA queue dispatch is
            # serialized ~18ns/descriptor + ~650ns issue, so fewer is faster)
            ina = bigp.tile([P, A_BYTES], U8, tag="ina")
            nc.sync.dma_start(out=ina[:, :], in_=ina_in[:, :])
            inb = bigp.tile([P, B_BYTES], U8, tag="inb")
            nc.scalar.dma_start(out=inb[:, :], in_=inb_in[:, :])

            cnt_bf = ina[:, 0:HPC * B].bitcast(FP8)  # fp8 to match M
            Mw = [ina[:, A_W0 + j * 2 * P:A_W0 + j * 2 * P + P].bitcast(FP8)
                  for j in range(HPC)]
            wv = [ina[:, A_W0 + j * 2 * P + P:A_W0 + j * 2 * P + 2 * P]
                  .bitcast(FP8) for j in range(HPC)]
            xt = [ina[:, A_XT0:A_XT0 + B * W].bitcast(FP8),
                  ina[:, A_XT1:A_XT1 + B * W].bitcast(FP8)]
            cf = ina[:, A_CF:A_CF + 16].bitcast(F32)
            cntT = [inb[:, B_CNTT + j * 4 * P:B_CNTT + (j + 1) * 4 * P]
                    .bitcast(F32) for j in range(HPC)]  # rows 0-7 used
            xn = [inb[:, B_XN0:B_XN0 + B * W].bitcast(FP8),
                  inb[:, B_XN1:B_XN1 + B * W].bitcast(FP8)]

            ones8 = spool.tile([P, 1], FP8, tag="ones8")
            nc.vector.memset(ones8[:, :], WVS)

            # ---- PE warmup while the DMAs land
            wdat = spool.tile([P, P], BF16, tag="warm")
            nc.vector.memset(wdat[:, :], 0.0)
            wps = psZO.tile([P, P], F32, tag="zo")
            for i in range(NWARM):
                nc.tensor.matmul(wps[:, :], wdat[:, :], wdat[:, :],
                                 start=True, stop=True)

            # ---- phase 1: kq for both heads (keeps DVE FIFO unblocked)
            kq_sb = []
            for j in range(HPC):
                kq_ps = psQK.tile([P, B], F32, tag="qk")
                nc.tensor.matmul(kq_ps[:, :], Mw[j], cnt_bf[:, j * B:(j + 1) * B],
                                 start=True, stop=True)
                k_sb = spool.tile([P, B], FP8, tag=f"kqsb{j}")
                nc.vector.tensor_scalar_add(k_sb[:, :], kq_ps[:, :],
                                            cf[:, 2 + j:3 + j])
                kq_sb.append(k_sb)

            # ---- phase 2: scores + exp, batched across heads per s-half
            # (both heads share the same per-partition bias for a given half,
            # so each half needs only ONE activation over [128,16])
            att = []
            for h in range(2):
                sch = psS.tile([P, HPC * B], F32, tag="sc")
                for j in range(HPC):
                    for b in range(B):
                        nc.tensor.matmul(sch[:, j * B + b:j * B + b + 1],
                                         xt[j][:, b * W + h * P:b * W + (h + 1) * P],
                                         kq_sb[j][:, b:b + 1],
                                         start=True, stop=True)
                a_h = spool.tile([P, HPC * B], FP8, tag=f"att{h}")
                nc.scalar.activation(a_h[:, :], sch[:, :],
                                     mybir.ActivationFunctionType.Exp,
                                     bias=cf[:, h:h + 1], scale=1.0 / KQS)
                att.append(a_h)

            # ---- phase 3: av + z^T; a_sb cols for head j live at j*B+b in
            # each half tile. Tails split across engines: h0 via GPSIMD, h1
            # via DVE, so the two normalize/residual chains run in parallel.
            av_sb, rec_t = [], []
            for j in range(HPC):
                pool = psQK if j == 0 else psS
                av = pool.tile([P, B], F32, tag="qk" if j == 0 else "sc")
                for b in range(B):
                    nc.tensor.matmul(av[:, b:b + 1],
                                     xn[j][:, b * W:b * W + P],
                                     att[0][:, j * B + b:j * B + b + 1],
                                     start=True, stop=False)
                    nc.tensor.matmul(av[:, b:b + 1],
                                     xn[j][:, b * W + P:b * W + 2 * P],
                                     att[1][:, j * B + b:j * B + b + 1],
                                     start=False, stop=True)
                zt_ps = pool.tile([B, 1], F32, tag="qk" if j == 0 else "sc")
                nc.tensor.matmul(zt_ps[:, :], att[0][:, j * B:(j + 1) * B],
                                 ones8, start=True, stop=False)
                nc.tensor.matmul(zt_ps[:, :], att[1][:, j * B:(j + 1) * B],
                                 ones8, start=False, stop=True)
                a_v = spool.tile([P, B], FP8, tag=f"avsb{j}")
                nc.vector.tensor_copy(a_v[:, :], av[:, :])
                r_t = spool.tile([B, 1], F32, tag=f"rec{j}")
                nc.vector.reciprocal(r_t[:, :], zt_ps[:, :])
                av_sb.append(a_v)
                rec_t.append(r_t)

            # ---- phase 4: o^T = (WVS*Wv)^T av, normalize by 1/(WVS*z), +cnt
            ot_ps = []
            for j in range(HPC):
                o_p = psZO.tile([B, P], F32, tag="zo")
                nc.tensor.matmul(o_p[:, :], av_sb[j][:, :], wv[j],
                                 start=True, stop=True)
                ot_ps.append(o_p)
            for j in range(HPC):
                tmp_t = spool.tile([B, P], F32, tag=f"tmp{j}")
                fin_t = spool.tile([B, P], F32, tag=f"fin{j}")
                if j == 0:
                    # h0 tail on ACT (PSUM-capable) + GPSIMD; h1 on DVE --
                    # the two chains run on disjoint engines in parallel
                    nc.scalar.activation(tmp_t[:, :], ot_ps[j][:, :],
                                         mybir.ActivationFunctionType.Copy,
                                         bias=0.0, scale=rec_t[j][:, :])
                    nc.gpsimd.tensor_add(fin_t[:, :], tmp_t[:, :],
                                         cntT[j][0:B, :])
                else:
                    nc.vector.tensor_scalar(tmp_t[:, :], ot_ps[j][:, :],
                                            rec_t[j][:, :], None,
                                            mybir.AluOpType.mult)
                    nc.vector.tensor_add(fin_t[:, :], tmp_t[:, :],
                                         cntT[j][0:B, :])
                if j == 0:
                    nc.scalar.dma_start(out=out_t[0], in_=fin_t[:, :])
                else:
                    nc.sync.dma_start(out=out_t[1], in_=fin_t[:, :])
    nc.finalize()
    return nc


def _get_nc():
    if "nc" not in _NC_CACHE:
        _NC_CACHE["nc"] = _build_nc()
    return _NC_CACHE["nc"]


def _pos_bias_f32():
    """t5_position_bucket exactly as the reference computes it, sliced to the
    window."""
    if "pos" not in _NC_CACHE:
        import jax.numpy as jnp
        NUM_BUCKETS, MAX_DISTANCE = 32, 128
        n = (S - 1) - jnp.arange(S)
        max_exact = NUM_BUCKETS // 2
        is_small = n < max_exact
        large = max_exact + (
            jnp.log(jnp.maximum(n, 1).astype(jnp.float32) / max_exact)
            / np.log(MAX_DISTANCE / max_exact)
            * (NUM_BUCKETS - max_exact)
        ).astype(jnp.int32)
        large = jnp.minimum(large, NUM_BUCKETS - 1)
        pos = jnp.where(is_small, n, large).astype(jnp.float32)
        _NC_CACHE["pos"] = np.asarray(pos)[CUTOFF:]  # [W]
    return _NC_CACHE["pos"]


def kernel(**inputs) -> np.ndarray:
    t = int(np.asarray(inputs["t"]))
    assert t == T, f"kernel hardcoded for t={T}, got {t}"
    content_t = np.asarray(inputs["content_t"], dtype=np.float32)
    cache = np.asarray(inputs["cache"], dtype=np.float32)
    Wq = np.asarray(inputs["Wq"], dtype=np.float32)
    bq = np.asarray(inputs["bq"], dtype=np.float32)
    Wk = np.asarray(inputs["Wk"], dtype=np.float32)
    Wv = np.asarray(inputs["Wv"], dtype=np.float32)
    bv = np.asarray(inputs["bv"], dtype=np.float32)
    pos_param = np.float32(np.asarray(inputs["pos_param"]))
    # time_mask: uniform -1e6 shift in-window (softmax-invariant); bk: adds an
    # s-constant q.bk to every in-window score (softmax-invariant). Both dropped.

    pos = _pos_bias_f32()                                   # [W]
    posb = (-pos_param * pos).astype(np.float32)            # [W]
    c = np.float32(KQS / np.sqrt(128.0))
    WVSf = np.float32(WVS)

    win = cache[:, CUTOFF:T, :].reshape(B, W - 1, H, P)     # [B, 255, H, 128]
    cnt_h = content_t.reshape(B, H, P)                      # [B, H, 128]

    in_maps = []
    for co in range(NCORES):
        h0 = HPC * co
        ina = np.zeros((P, A_BYTES), np.uint8)
        inb = np.zeros((P, B_BYTES), np.uint8)
        ina[:, 0:HPC * B] = (
            cnt_h[:, h0:h0 + HPC, :].transpose(2, 1, 0).reshape(P, HPC * B)
            .astype(NPF8).view(np.uint8)
        )
        for j in range(HPC):
            base = A_W0 + j * 2 * P
            h = h0 + j
            ina[:, base:base + P] = (
                (c * (Wq[h] @ Wk[h].T)).astype(NPF8).view(np.uint8)
            )
            ina[:, base + P:base + 2 * P] = (
                (np.float32(WVS) * Wv[h]).astype(NPF8).view(np.uint8)
            )
        for j in range(HPC):
            xwin = np.concatenate(
                [win[:, :, h0 + j, :], cnt_h[:, None, h0 + j, :]], axis=1
            )                                               # [B, 256, 128] f32
            xwb = xwin.astype(NPF8)
            off = A_XT0 if j == 0 else A_XT1
            ina[:, off:off + B * W] = (
                xwb.transpose(2, 0, 1).reshape(P, B * W).view(np.uint8)
            )
            offn = B_XN0 if j == 0 else B_XN1
            inb[:, offn:offn + B * W] = (
                xwb.reshape(B, 2, P, P).transpose(2, 0, 1, 3)
                .reshape(P, B * W).view(np.uint8)
            )
        cfh = np.zeros((P, CFW), np.float32)
        cfh[:, 0] = posb[0:P]
        cfh[:, 1] = posb[P:2 * P]
        for j in range(HPC):
            cfh[:, 2 + j] = c * (Wk[h0 + j] @ bq[h0 + j])
        ina[:, A_CF:A_CF + 16] = cfh.view(np.uint8)
        for j in range(HPC):
            cntT = np.zeros((P, P), np.float32)
            cntT[0:B, :] = cnt_h[:, h0 + j, :] + bv[None, h0 + j, :]
            inb[:, B_CNTT + j * 4 * P:B_CNTT + (j + 1) * 4 * P] = (
                cntT.view(np.uint8)
            )
        in_maps.append({"ina": ina, "inb": inb})

    nc = _get_nc()
    res = run_bass_kernel_spmd(nc, in_maps, list(range(NCORES)), **_RUN_KWARGS)
    _NC_CACHE["last_results"] = res
    outs = np.stack([np.asarray(res.results[co]["out"]) for co in range(NCORES)])
    # outs: [core, j, b, d] -> out_full[b, (2c+j)*128 + d]
    out_full = outs.transpose(2, 0, 1, 3).reshape(B, H * P)
    return np.ascontiguousarray(out_full, dtype=np.float32)


_RUN_KWARGS = {}  # test harness may set {"trace": True, "tmpdir": ...}
